# revision 12
# baseline (speedup 1.0000x reference)
"""Trainium2 Bass kernel for the temporal-shift multi-head attention module.

Sharding: data-parallel over the video axis — 8 videos of 8 frames each,
one video (8 frames x 197 tokens) per NeuronCore. The temporal head shift
only moves data between frames of the same video, so it is a pure slicing
operation on-device. Weights are replicated. No collectives.

Per-core pipeline (all on-chip, bf16 matmul operands, fp32 accumulation):
  1. DMA x naturally, cast bf16, PE-transpose to xT [C(part), M(free)].
  2. Adapter: hT = aw1^T @ xT (+b1); xT += aw2^T @ hT (+b2)  (in-place x1).
  3. qT/kT = W^T @ x1T (channel-major, ACT bias-evict); v = x1T^T @ Wv
     (token-major per frame, DVE bias-evict).
  4. Attention frame loop, 6 units per frame (unit p = head pair
     (2p, 2p+1)), software-pipelined for PE density (HAM must stay
     un-throttled: any PE idle gap >~1.7us halves the PE clock):
     - scores for pair p go into one 2-bank PSUM tile (odd head at bank
       offsets 0/512 — base-64-partition matmuls must write bank-aligned
       offsets — even head in-bank at 197/709), then ONE batched exp per
       pair via a 3D strided AP -> es [128, 4*197] bf16.
     - av matmuls run IN-frame right after exp p, unnormalized, evicted
       to SBUF bf16; the normalization multiply trails into frame f+1
       after the reciprocal bounce, so the PE never waits on it.
     - denominators: one N=394 matmul per (pair, key chunk) through the
       onesel selector into a [6, 394] PSUM bank, one batched DVE
       reciprocal per frame, then a DMA bounce through DRAM with TWO
       broadcast descriptors into rec128 [128, 6*197] (rows 0:64 = even
       heads, 64:128 = odd heads).
  5. proj: both halves of out = aoT^T @ Wp in one 2-bank PSUM tile, one
     batched DVE bias-add, DMA out; tile m is emitted as soon as all
     frames its token rows touch have normalized aoT.

Hardware pitfalls encoded here: matmul weight APs must have one free dim;
matmuls from base-partition-64 operands must not write offset sub-regions
of a PSUM bank (bank-aligned offsets are fine); DMA cannot read PSUM;
SBUF APs cannot partition-broadcast (DRAM sources can); custom-DVE ops
(reciprocal_approx_*) crash this runtime.
"""

import numpy as np

F = 8
N = 197
C = 768
HADP = 192
NH = 12
HD = 64
M = F * N  # 1576
SCALE = HD ** -0.5
NCORES = 8
MCHUNKS = [(0, 512), (512, 512), (1024, 512), (1536, 40)]
MTILES = [(i * 128, 128) for i in range(12)] + [(1536, 40)]
JTILES = [(0, 128), (128, 69)]

_CACHE = {}


def _build():
    import contextlib

    @contextlib.contextmanager
    def pst_ctx_closer(ctx):
        try:
            yield
        finally:
            ctx.__exit__(None, None, None)

    import concourse.mybir as mybir
    from concourse import bacc
    import concourse.tile as tile
    from concourse.masks import make_identity

    BF = mybir.dt.bfloat16
    FP = mybir.dt.float32
    AT = mybir.ActivationFunctionType
    OP = mybir.AluOpType

    nc = bacc.Bacc("TRN2", target_bir_lowering=False, debug=False)

    x_e = nc.dram_tensor("x", [F, N, C], FP, kind="ExternalInput")
    aw1_e = nc.dram_tensor("a_w1", [C, HADP], FP, kind="ExternalInput")
    ab1_e = nc.dram_tensor("a_b1", [HADP], FP, kind="ExternalInput")
    aw2_e = nc.dram_tensor("a_w2", [HADP, C], FP, kind="ExternalInput")
    ab2_e = nc.dram_tensor("a_b2", [C], FP, kind="ExternalInput")
    qkvw_e = nc.dram_tensor("qkv_w", [C, 3 * C], FP, kind="ExternalInput")
    qkvb_e = nc.dram_tensor("qkv_b", [3 * C], FP, kind="ExternalInput")
    projw_e = nc.dram_tensor("proj_w", [C, C], FP, kind="ExternalInput")
    projb_e = nc.dram_tensor("proj_b", [C], FP, kind="ExternalInput")
    out_e = nc.dram_tensor("out", [F, N, C], FP, kind="ExternalOutput")

    xf = x_e.rearrange("f n c -> (f n) c")
    outf = out_e.rearrange("f n c -> (f n) c")

    with tile.TileContext(nc) as tc:
        with tc.tile_pool(name="persist", bufs=1) as pp, \
             tc.tile_pool(name="scratch", bufs=2) as sp:
            # ---- constants
            ident = pp.tile([128, 128], BF, name="ident", tag="ident")
            make_identity(nc, ident)

            # ---- persistent activations
            xT = [pp.tile([128, M], BF, name=f"xT{i}", tag=f"xT{i}") for i in range(6)]
            qT = [pp.tile([128, M], BF, name=f"qT{i}", tag=f"qT{i}") for i in range(6)]
            # kT is padded by 64 zeroed columns: the frame loop always runs
            # the second key chunk with a full 128-wide lhsT window (so score
            # matmuls write all 128 PSUM partitions and the batched exp never
            # reads unwritten memory); for the last frame that window hangs
            # 59 columns past M.
            kT = [pp.tile([128, M + 64], BF, name=f"kT{i}", tag=f"kT{i}")
                  for i in range(6)]
            for i in range(6):
                nc.vector.memset(kT[i][:, M:M + 64], 0.0)
            aoT = [pp.tile([128, M], BF, name=f"aoT{i}", tag=f"aoT{i}") for i in range(6)]
            # v per frame/token-tile, natural token-major layout [tok, chan]
            vt = [[pp.tile([128, C], BF, name=f"v{f}_{j}", tag=f"v{f}_{j}")
                   for j in range(2)] for f in range(F)]
            # pair-selector blocks for the denominator matmuls: block p
            # (cols 6p:6p+6) is all-ones in column p, zero elsewhere, so a
            # single N=394 matmul with rhs = both heads of pair p writes
            # their key-sums into row p of a shared [6, 394] PSUM tile
            # (other rows accumulate +0).
            onesel = pp.tile([128, 6 * 6], BF, name="onesel", tag="onesel")
            nc.vector.memset(onesel[:, :], 0.0)
            for p in range(6):
                nc.vector.memset(onesel[:, p * 6 + p:p * 6 + p + 1], 1.0)

            # ---- phases 1-4, m-chunk pipelined: weight DMAs stream on the
            # otherwise-idle GpSimd queue (column-chunk-major so the qk
            # matmuls of chunk 0 can start after ~1/3 of qkv_w has landed)
            # while x loads/transposes and adapter/qk/v compute proceed per
            # 512-token chunk on sync/PE/ACT/DVE. This overlaps the whole
            # weight-load latency with compute instead of serializing.
            # chunk-0 x tiles are emitted before any weight loads so the
            # x casts/transposes head every engine queue (weight casts
            # otherwise block them in-order and delay the first matmul)
            pst_ctx = tc.tile_pool(name="pst", bufs=2, space="PSUM")
            pst = pst_ctx.__enter__()

            def emit_x_tile(mt):
                tb, tsz = MTILES[mt]
                xn = sp.tile([128, C], FP, bufs=4, name=f"xn{mt}", tag="wstg")
                nc.sync.dma_start(xn[0:tsz, :], xf[tb:tb + tsz, :])
                xb = sp.tile([128, C], BF, bufs=3, name=f"xb{mt}", tag="xb")
                nc.vector.tensor_copy(xb[0:tsz, :], xn[0:tsz, :])
                for ct in range(6):
                    pt = pst.tile([128, 128], BF, name=f"pt{mt}_{ct}", tag="pt")
                    nc.tensor.transpose(pt[:, 0:tsz],
                                        xb[0:tsz, ct * 128:(ct + 1) * 128],
                                        ident[0:tsz, 0:tsz])
                    nc.scalar.activation(xT[ct][:, tb:tb + tsz],
                                         pt[:, 0:tsz], AT.Copy)

            for mt in (0, 1, 2, 3):
                emit_x_tile(mt)

            _castn = [0]

            def stage_cast(dst, cb, csz, p, src_ap, name, q=None):
                stg = sp.tile([128, C], FP, bufs=4,
                              name=f"stg_{name}_{cb}", tag="wstg")
                # spread DMA issue between the sync and gpsimd queues
                # (issue costs ~0.7us each; one queue would serialize the
                # whole weight stream), and alternate the cast between DVE
                # and ACT
                if q is None:
                    q = nc.sync if _castn[0] % 2 == 0 else nc.gpsimd
                q.dma_start(stg[0:p, 0:csz], src_ap[:, cb:cb + csz])
                if _castn[0] % 2 == 0:
                    nc.vector.tensor_copy(dst[:, cb:cb + csz], stg[0:p, 0:csz])
                else:
                    nc.scalar.copy(dst[:, cb:cb + csz], stg[0:p, 0:csz])
                _castn[0] += 1

            aw1 = [pp.tile([128, HADP], BF, name=f"aw1_{k}", tag=f"aw1_{k}")
                   for k in range(6)]
            aw2 = [pp.tile([128, C], BF, name="aw2_0", tag="aw2_0"),
                   pp.tile([64, C], BF, name="aw2_1", tag="aw2_1")]
            qkvw = [pp.tile([128, 3 * C], BF, name=f"qkvw{k}", tag=f"qkvw{k}")
                    for k in range(6)]
            projw = [pp.tile([128, C], BF, name=f"projw{k}", tag=f"projw{k}")
                     for k in range(6)]

            # adapter biases first (needed ~5us in); packed column loads
            # (one DMA per bias tensor instead of one per 128-column)
            b1c_t = pp.tile([128, 2], FP, name="b1c", tag="b1c")
            nc.gpsimd.dma_start(b1c_t[:, 0:1], ab1_e[0:128][:, None])
            nc.gpsimd.dma_start(b1c_t[0:64, 1:2], ab1_e[128:HADP][:, None])
            b1c = [b1c_t[:, 0:1], b1c_t[0:64, 1:2]]
            b2c_t = pp.tile([128, 6], FP, name="b2c", tag="b2c")
            nc.gpsimd.dma_start(
                b2c_t[:, :], ab2_e[:].rearrange("(a p) -> p a", p=128))
            b2c = [b2c_t[:, i:i + 1] for i in range(6)]
            # qkv biases: first 12 columns are q/k (per-partition columns),
            # v bias needs the row-broadcast layout
            qkc_t = pp.tile([128, NH], FP, name="qkc", tag="qkc")
            nc.gpsimd.dma_start(
                qkc_t[:, :], qkvb_e[0:1536].rearrange("(a p) -> p a", p=128))
            qkbc = [qkc_t[:, i:i + 1] for i in range(12)]
            vbb = pp.tile([128, C], FP, name="vbb", tag="vbb")
            pbb = pp.tile([128, C], FP, name="pbb", tag="pbb")

            def load_qkvw_cc(cc):
                for k in range(6):
                    stage_cast(qkvw[k], cc * C, C,
                               128, qkvw_e[k * 128:(k + 1) * 128, :],
                               f"qkvw{k}")

            # adapter weights first (first compute consumer; small), then
            # the q/k/v weight column-chunks in first-use order
            for k in range(6):
                stage_cast(aw1[k], 0, HADP, 128, aw1_e[k * 128:(k + 1) * 128, :],
                           f"aw1_{k}", q=nc.gpsimd)
            stage_cast(aw2[0], 0, C, 128, aw2_e[0:128, :], "aw2_0", q=nc.gpsimd)
            stage_cast(aw2[1], 0, C, 64, aw2_e[128:HADP, :], "aw2_1",
                       q=nc.gpsimd)
            load_qkvw_cc(0)
            load_qkvw_cc(1)
            load_qkvw_cc(2)

            CHUNK_MTILES = {0: [0, 1, 2, 3], 1: [4, 5, 6, 7],
                            2: [8, 9, 10, 11], 3: [12]}
            CHUNK_FRAMES = {0: [0, 1], 1: [2, 3, 4], 2: [5, 6], 3: [7]}

            with pst_ctx_closer(pst_ctx), \
                 tc.tile_pool(name="psA", bufs=4, space="PSUM") as psA:
                hT = [sp.tile([128, M], BF, bufs=1, name="hT0", tag="hT0"),
                      sp.tile([64, M], BF, bufs=1, name="hT1", tag="hT1")]
                for c, (mb, csz) in enumerate(MCHUNKS):
                    # adapter for this chunk (m-local).  x tiles for chunk
                    # c+1 are emitted right after the adapter so their
                    # DMAs/casts/transposes prefetch during this chunk's
                    # qk/v compute instead of stalling the next chunk.
                    for ht, (hb, hsz) in enumerate([(0, 128), (128, 64)]):
                        ps = psA.tile([128, 512], FP, name=f"psh{ht}_{mb}",
                                      tag="psA")
                        for kt in range(6):
                            nc.tensor.matmul(ps[0:hsz, 0:csz],
                                             aw1[kt][:, hb:hb + hsz],
                                             xT[kt][:, mb:mb + csz],
                                             start=(kt == 0), stop=(kt == 5))
                        nc.scalar.activation(hT[ht][:, mb:mb + csz],
                                             ps[0:hsz, 0:csz],
                                             AT.Identity, bias=b1c[ht][:, :])
                    for ct in range(6):
                        ps = psA.tile([128, 512], FP, name=f"psx{ct}_{mb}",
                                      tag="psA")
                        for kt, ksz in enumerate([128, 64]):
                            nc.tensor.matmul(ps[:, 0:csz],
                                             aw2[kt][0:ksz, ct * 128:(ct + 1) * 128],
                                             hT[kt][0:ksz, mb:mb + csz],
                                             start=(kt == 0), stop=(kt == 1))
                        nc.vector.scalar_tensor_tensor(
                            out=xT[ct][:, mb:mb + csz], in0=ps[:, 0:csz],
                            scalar=b2c[ct][:, :], in1=xT[ct][:, mb:mb + csz],
                            op0=OP.add, op1=OP.add)
                    if c + 1 < len(MCHUNKS):
                        for mt in CHUNK_MTILES[c + 1]:
                            emit_x_tile(mt)
                    # qk for this chunk
                    for ot in range(12):
                        dst = qT[ot] if ot < 6 else kT[ot - 6]
                        ps = psA.tile([128, 512], FP, name=f"psqk{ot}_{mb}",
                                      tag="psA")
                        for kt in range(6):
                            nc.tensor.matmul(ps[:, 0:csz],
                                             qkvw[kt][:, ot * 128:(ot + 1) * 128],
                                             xT[kt][:, mb:mb + csz],
                                             start=(kt == 0), stop=(kt == 5))
                        nc.scalar.activation(dst[:, mb:mb + csz], ps[:, 0:csz],
                                             AT.Identity, bias=qkbc[ot][:, :])
                    if c == 0:
                        nc.gpsimd.dma_start(
                            vbb[:, :],
                            qkvb_e[2 * C:3 * C][None, :].broadcast_to((128, C)))
                    # v for the frames fully inside chunks <= c
                    for f in CHUNK_FRAMES[c]:
                        for jt, (jb, jsz) in enumerate(JTILES):
                            for half in range(2):
                                ps = psA.tile([128, 512], FP,
                                              name=f"psv{f}_{jt}_{half}",
                                              tag="psA")
                                for kt in range(6):
                                    nc.tensor.matmul(
                                        ps[0:jsz, 0:384],
                                        xT[kt][:, f * N + jb: f * N + jb + jsz],
                                        qkvw[kt][:, 1536 + half * 384:
                                                 1536 + (half + 1) * 384],
                                        start=(kt == 0), stop=(kt == 5))
                                nc.vector.tensor_tensor(
                                    out=vt[f][jt][0:jsz,
                                                  half * 384:(half + 1) * 384],
                                    in0=ps[0:jsz, 0:384],
                                    in1=vbb[0:jsz, half * 384:(half + 1) * 384],
                                    op=OP.add)
                    if c == 1:
                        nc.gpsimd.dma_start(
                            pbb[:, :],
                            projb_e[:][None, :].broadcast_to((128, C)))
                        for k in range(6):
                            stage_cast(projw[k], 0, C,
                                       128, projw_e[k * 128:(k + 1) * 128, :],
                                       f"projw{k}")

            # ---- phase 5: attention frame loop, 9 score units per
            # frame (3 'E' tiles: both key chunks of even heads (2a, 2a+6)
            # in one 2-bank tile at in-bank offsets — legal for base-0
            # operands; 6 'O' tiles: one key chunk of odd heads
            # (2c+1, 2c+7) at bank-aligned offsets — base-64 matmuls must
            # write bank offset 0 and banks must not mix base-64/base-0
            # writers), one batched exp per unit.
            # av matmuls for pair p run IN-frame as soon as its three es
            # units exist (unnormalized, evicted to SBUF bf16 on DVE); the
            # normalization multiply trails into frame f+1 after the
            # reciprocal DRAM bounce, so neither the PE nor the frame
            # cadence ever waits on the bounce.
            # den: one N=394 matmul per (unit, key chunk) — lhsT is an
            # all-ones column selecting one row of a [6, 394] PSUM tile
            # (row a = even pair (2a, 2a+6), row 3+c = odd pair
            # (2c+1, 2c+7); cols 0:197 first head, 197:394 second).
            def fk_of(f, h):
                if h < 2:
                    return max(f - 1, 0)
                if h < 4:
                    return min(f + 1, F - 1)
                return f

            # proj tile m is emitted during the frame after the last frame
            # its token rows touch; PROJ_AT[f] = tiles ready in frame f
            PROJ_AT = {1: [0], 2: [1, 2], 3: [3], 4: [4, 5], 5: [6],
                       6: [7, 8], 7: [9]}
            UNITS = [("E", 0), ("O", 0, 0), ("O", 0, 1),
                     ("E", 1), ("O", 1, 0), ("O", 1, 1),
                     ("E", 2), ("O", 2, 0), ("O", 2, 1)]
            # unit index -> av pairs whose es units are all ready
            AV_AT = {3: [0], 4: [3], 6: [1], 7: [4]}

            def unit_heads(u):
                # [(head, jt, bank_col)] covered by unit u
                if u[0] == "E":
                    a = u[1]
                    return [(2 * a + 6 * s, jt, s * 512 + jt * N)
                            for s in range(2) for jt in range(2)]
                _, c, jt = u
                return [(2 * c + 1 + 6 * s, jt, s * 512) for s in range(2)]

            with tc.tile_pool(name="psT", bufs=1, space="PSUM") as psT, \
                 tc.tile_pool(name="drp", bufs=2, space="DRAM") as drp:

                es_all = {}       # es_all[(f, u)] -> es tile
                aou_all = {}      # aou_all[(f, p)] -> [128, 197] bf16 unnorm
                rec128_all = {}   # rec128_all[f] -> [128, 6*197] fp32
                den_all = {}      # den_all[f] -> [6, 512] PSUM
                nden = {}

                def es_get(gf, h, jt):
                    if h % 2 == 0:
                        a = (h // 2) % 3
                        return (es_all[(gf, ("E", a))],
                                (0 if h < 6 else 2 * N) + jt * N)
                    c = ((h - 1) // 2) % 3
                    return es_all[(gf, ("O", c, jt))], (0 if h < 7 else N)

                def emit_scores(f, i, u):
                    esz = 4 * N if u[0] == "E" else 2 * N
                    stt = psT.tile([128, 1024], FP, bufs=2,
                                   name=f"st{f}_{i}", tag="st")
                    for h, jt, bcol in unit_heads(u):
                        jb = JTILES[jt][0]
                        pb = 64 * (h % 2)
                        g = h // 2
                        fk = fk_of(f, h)
                        # always a full 128-wide kT window so all PSUM
                        # partitions the batched exp reads are written
                        # (the jt=1 tail rows hit the next frame's keys /
                        # the zero pad; never read downstream)
                        nc.tensor.matmul(
                            stt[0:128, bcol:bcol + N],
                            kT[g][pb:pb + 64,
                                  fk * N + jb: fk * N + jb + 128],
                            qT[g][pb:pb + 64, f * N:(f + 1) * N],
                            start=True, stop=True)
                    e = sp.tile([128, esz], BF,
                                bufs=(4 if u[0] == "E" else 7),
                                name=f"e{f}_{i}",
                                tag=("eE" if u[0] == "E" else "eO"))
                    nc.scalar.activation(
                        e[0:128, :].rearrange("p (b q) -> p b q", b=2),
                        stt[0:128, 0:1024].rearrange(
                            "p (b q) -> p b q", b=2)[:, :, 0:esz // 2],
                        AT.Exp, scale=SCALE)
                    es_all[(f, u)] = e

                def emit_den(f, u):
                    if f not in den_all:
                        den_all[f] = psT.tile([6, 512], FP, bufs=1,
                                              name=f"den{f}", tag="den")
                        nden[f] = 0
                    dt = den_all[f]
                    e = es_all[(f, u)]
                    if u[0] == "E":
                        a = u[1]
                        for jt, (jb, jsz) in enumerate(JTILES):
                            nc.tensor.matmul(
                                dt[0:6, 0:2 * N],
                                onesel[0:jsz, a * 6:(a + 1) * 6],
                                e[0:jsz, :].rearrange(
                                    "p (hh q) -> p hh q",
                                    hh=2)[:, :, jt * N:(jt + 1) * N],
                                start=(nden[f] == 0), stop=(nden[f] == 11))
                            nden[f] += 1
                    else:
                        c, jt = u[1], u[2]
                        jsz = JTILES[jt][1]
                        nc.tensor.matmul(
                            dt[0:6, 0:2 * N],
                            onesel[0:jsz, (3 + c) * 6:(4 + c) * 6],
                            e[0:jsz, 0:2 * N],
                            start=(nden[f] == 0), stop=(nden[f] == 11))
                        nden[f] += 1

                def emit_av(f, p):
                    av_t = psT.tile([128, 512], FP, bufs=2,
                                    name=f"av{f}_{p}", tag="av")
                    for hi in range(2):
                        h = 2 * p + hi
                        fk = fk_of(f, h)
                        for jt, (jb, jsz) in enumerate(JTILES):
                            e, cb = es_get(f, h, jt)
                            nc.tensor.matmul(
                                av_t[hi * 64:(hi + 1) * 64, 0:N],
                                vt[fk][jt][0:jsz, h * HD:(h + 1) * HD],
                                e[0:jsz, cb:cb + N],
                                start=(jt == 0), stop=(jt == 1))
                    ao = sp.tile([128, N], BF, bufs=8, name=f"aou{f}_{p}",
                                 tag="aou")
                    nc.vector.tensor_copy(ao[:, :], av_t[:, 0:N])
                    aou_all[(f, p)] = ao

                def emit_norm(f, p):
                    nc.vector.tensor_tensor(
                        out=aoT[p][:, f * N:(f + 1) * N],
                        in0=aou_all.pop((f, p))[:, :],
                        in1=rec128_all[f][:, p * N:(p + 1) * N],
                        op=OP.mult)

                def emit_bounce(f):
                    rec6 = sp.tile([6, 2 * N], FP, bufs=2, name=f"rcp{f}",
                                   tag="rec6")
                    nc.vector.reciprocal(rec6[:, :], den_all[f][:, 0:2 * N])
                    dr6 = drp.tile([6, 2 * N], FP, name=f"dr6_{f}", tag="dr6")
                    nc.sync.dma_start(dr6[:, :], rec6[:, :])
                    rec128 = sp.tile([128, 6 * N], FP, bufs=2,
                                     name=f"rec128_{f}", tag="rec128")
                    for b in range(2):
                        # b=0: rows 0:64 <- even heads = dr6 rows 0:3;
                        # b=1: rows 64:128 <- odd heads = dr6 rows 3:6.
                        # rec128 col-block p maps to (half a, den row r)
                        # with p = a*3 + r
                        for a in range(2):
                            src = dr6[3 * b:3 * b + 3,
                                      a * N:(a + 1) * N][None]
                            nc.sync.dma_start(
                                rec128[b * 64:(b + 1) * 64,
                                       a * 3 * N:(a + 1) * 3 * N].rearrange(
                                    "p (r q) -> p r q", r=3),
                                src.broadcast_to((64, 3, N)))
                    rec128_all[f] = rec128

                def emit_proj(mt):
                    mb, msz = MTILES[mt]
                    pt = psT.tile([128, 1024], FP, bufs=2,
                                  name=f"psp{mt}", tag="st")
                    for half in range(2):
                        for kt in range(6):
                            nc.tensor.matmul(
                                pt[0:msz, half * 512:half * 512 + 384],
                                aoT[kt][:, mb:mb + msz],
                                projw[kt][:, half * 384:(half + 1) * 384],
                                start=(kt == 0), stop=(kt == 5))
                    osb = sp.tile([128, C], FP, bufs=2,
                                  name=f"osb{mt}", tag="osb")
                    nc.vector.tensor_tensor(
                        out=osb[0:msz, :].rearrange("p (b c) -> p b c", b=2),
                        in0=pt[0:msz, 0:1024].rearrange(
                            "p (b c) -> p b c", b=2)[:, :, 0:384],
                        in1=pbb[0:msz, :].rearrange("p (b c) -> p b c", b=2),
                        op=OP.add)
                    nc.sync.dma_start(outf[mb:mb + msz, :], osb[0:msz, :])

                for f in range(F):
                    for i, u in enumerate(UNITS):
                        emit_scores(f, i, u)
                        if i >= 1:
                            emit_den(f, UNITS[i - 1])
                        for pq in AV_AT.get(i, ()):
                            emit_av(f, pq)
                        if f >= 1:
                            if i == 2:
                                for q in range(3):
                                    emit_norm(f - 1, q)
                            if i == 4:
                                for q in range(3, 6):
                                    emit_norm(f - 1, q)
                            if i == 5 and f in PROJ_AT:
                                emit_proj(PROJ_AT[f][0])
                            if i == 7 and len(PROJ_AT.get(f, ())) > 1:
                                emit_proj(PROJ_AT[f][1])
                    emit_den(f, UNITS[8])
                    emit_av(f, 2)
                    emit_av(f, 5)
                    emit_bounce(f)

                # epilogue: frame 7 normalization trails the bounce; proj
                # tile 10's rows 1280:1344 only need frames <= 6, so those
                # matmuls bridge the frame-7 reciprocal-bounce latency
                mb10 = MTILES[10][0]
                pt10 = psT.tile([128, 1024], FP, bufs=2,
                                name="psp10", tag="st")
                for half in range(2):
                    for kt in range(6):
                        nc.tensor.matmul(
                            pt10[0:64, half * 512:half * 512 + 384],
                            aoT[kt][:, mb10:mb10 + 64],
                            projw[kt][:, half * 384:(half + 1) * 384],
                            start=(kt == 0), stop=(kt == 5))
                for q in range(6):
                    emit_norm(7, q)
                for half in range(2):
                    for kt in range(6):
                        nc.tensor.matmul(
                            pt10[64:128, half * 512:half * 512 + 384],
                            aoT[kt][:, mb10 + 64:mb10 + 128],
                            projw[kt][:, half * 384:(half + 1) * 384],
                            start=(kt == 0), stop=(kt == 5))
                osb10 = sp.tile([128, C], FP, bufs=2, name="osb10", tag="osb")
                nc.vector.tensor_tensor(
                    out=osb10[0:128, :].rearrange("p (b c) -> p b c", b=2),
                    in0=pt10[0:128, 0:1024].rearrange(
                        "p (b c) -> p b c", b=2)[:, :, 0:384],
                    in1=pbb[0:128, :].rearrange("p (b c) -> p b c", b=2),
                    op=OP.add)
                nc.sync.dma_start(outf[mb10:mb10 + 128, :], osb10[0:128, :])
                emit_proj(11)
                emit_proj(12)

    nc.compile()
    return nc


def _get_nc():
    if "nc" not in _CACHE:
        _CACHE["nc"] = _build()
    return _CACHE["nc"]


def _in_maps(inputs):
    x = np.ascontiguousarray(np.asarray(inputs["x"], np.float32))
    w = {k: np.ascontiguousarray(np.asarray(inputs[k], np.float32))
         for k in ("a_w1", "a_b1", "a_w2", "a_b2", "qkv_w", "qkv_b",
                   "proj_w", "proj_b")}
    maps = []
    for i in range(NCORES):
        m = {"x": x[i * F:(i + 1) * F]}
        m.update(w)
        maps.append(m)
    return maps


def kernel(**inputs):
    from concourse.bass_utils import run_bass_kernel_spmd
    nc = _get_nc()
    res = run_bass_kernel_spmd(nc, _in_maps(inputs), core_ids=list(range(NCORES)))
    return np.concatenate([res.results[i]["out"] for i in range(NCORES)], axis=0)


def run_traced(inputs, **kwargs):
    """Test harness helper: run with NTFF profiling, return (output, results)."""
    from concourse.bass_utils import run_bass_kernel_spmd
    nc = _get_nc()
    res = run_bass_kernel_spmd(nc, _in_maps(inputs),
                               core_ids=list(range(NCORES)), trace=True, **kwargs)
    out = np.concatenate([res.results[i]["out"] for i in range(NCORES)], axis=0)
    return out, res


if __name__ == "__main__":
    # quick compile check
    _build()
    print("compile OK")



# revision 13
# speedup vs baseline: 1.0534x; 1.0534x over previous
"""Trainium2 Bass kernel for the temporal-shift multi-head attention module.

Sharding: data-parallel over the video axis — 8 videos of 8 frames each,
one video (8 frames x 197 tokens) per NeuronCore. The temporal head shift
only moves data between frames of the same video, so it is a pure slicing
operation on-device. Weights are replicated. No collectives.

Per-core pipeline (all on-chip, bf16 matmul operands, fp32 accumulation):
  1. DMA x naturally, cast bf16, PE-transpose to xT [C(part), M(free)].
  2. Adapter: hT = aw1^T @ xT (+b1); xT += aw2^T @ hT (+b2)  (in-place x1).
  3. qT/kT = W^T @ x1T (channel-major, ACT bias-evict); v = x1T^T @ Wv
     (token-major per frame, DVE bias-evict).
  4. Attention frame loop, 6 units per frame (unit p = head pair
     (2p, 2p+1)), software-pipelined for PE density (HAM must stay
     un-throttled: any PE idle gap >~1.7us halves the PE clock):
     - scores for pair p go into one 2-bank PSUM tile (odd head at bank
       offsets 0/512 — base-64-partition matmuls must write bank-aligned
       offsets — even head in-bank at 197/709), then ONE batched exp per
       pair via a 3D strided AP -> es [128, 4*197] bf16.
     - av matmuls run IN-frame right after exp p, unnormalized, evicted
       to SBUF bf16; the normalization multiply trails into frame f+1
       after the reciprocal bounce, so the PE never waits on it.
     - denominators: one N=394 matmul per (pair, key chunk) through the
       onesel selector into a [6, 394] PSUM bank, one batched DVE
       reciprocal per frame, then a DMA bounce through DRAM with TWO
       broadcast descriptors into rec128 [128, 6*197] (rows 0:64 = even
       heads, 64:128 = odd heads).
  5. proj: both halves of out = aoT^T @ Wp in one 2-bank PSUM tile, one
     batched DVE bias-add, DMA out; tile m is emitted as soon as all
     frames its token rows touch have normalized aoT.

Hardware pitfalls encoded here: matmul weight APs must have one free dim;
matmuls from base-partition-64 operands must not write offset sub-regions
of a PSUM bank (bank-aligned offsets are fine); DMA cannot read PSUM;
SBUF APs cannot partition-broadcast (DRAM sources can); custom-DVE ops
(reciprocal_approx_*) crash this runtime.
"""

import numpy as np

F = 8
N = 197
C = 768
HADP = 192
NH = 12
HD = 64
M = F * N  # 1576
SCALE = HD ** -0.5
NCORES = 8
MCHUNKS = [(0, 512), (512, 512), (1024, 512), (1536, 40)]
MTILES = [(i * 128, 128) for i in range(12)] + [(1536, 40)]
JTILES = [(0, 128), (128, 69)]

_CACHE = {}


def _build():
    import contextlib

    @contextlib.contextmanager
    def pst_ctx_closer(ctx):
        try:
            yield
        finally:
            ctx.__exit__(None, None, None)

    import concourse.mybir as mybir
    from concourse import bacc
    import concourse.tile as tile
    from concourse.masks import make_identity

    BF = mybir.dt.bfloat16
    FP = mybir.dt.float32
    AT = mybir.ActivationFunctionType
    OP = mybir.AluOpType

    nc = bacc.Bacc("TRN2", target_bir_lowering=False, debug=False)

    x_e = nc.dram_tensor("x", [F, N, C], FP, kind="ExternalInput")
    aw1_e = nc.dram_tensor("a_w1", [C, HADP], FP, kind="ExternalInput")
    ab1_e = nc.dram_tensor("a_b1", [HADP], FP, kind="ExternalInput")
    aw2_e = nc.dram_tensor("a_w2", [HADP, C], FP, kind="ExternalInput")
    ab2_e = nc.dram_tensor("a_b2", [C], FP, kind="ExternalInput")
    qkvw_e = nc.dram_tensor("qkv_w", [C, 3 * C], FP, kind="ExternalInput")
    qkvb_e = nc.dram_tensor("qkv_b", [3 * C], FP, kind="ExternalInput")
    projw_e = nc.dram_tensor("proj_w", [C, C], FP, kind="ExternalInput")
    projb_e = nc.dram_tensor("proj_b", [C], FP, kind="ExternalInput")
    out_e = nc.dram_tensor("out", [F, N, C], FP, kind="ExternalOutput")

    xf = x_e.rearrange("f n c -> (f n) c")
    outf = out_e.rearrange("f n c -> (f n) c")

    with tile.TileContext(nc) as tc:
        with tc.tile_pool(name="persist", bufs=1) as pp, \
             tc.tile_pool(name="scratch", bufs=2) as sp:
            # ---- constants
            ident = pp.tile([128, 128], BF, name="ident", tag="ident")
            make_identity(nc, ident)

            # ---- persistent activations
            xT = [pp.tile([128, M], BF, name=f"xT{i}", tag=f"xT{i}") for i in range(6)]
            qT = [pp.tile([128, M], BF, name=f"qT{i}", tag=f"qT{i}") for i in range(6)]
            # kT is padded by 64 zeroed columns: the frame loop always runs
            # the second key chunk with a full 128-wide lhsT window (so score
            # matmuls write all 128 PSUM partitions and the batched exp never
            # reads unwritten memory); for the last frame that window hangs
            # 59 columns past M.
            kT = [pp.tile([128, M + 64], BF, name=f"kT{i}", tag=f"kT{i}")
                  for i in range(6)]
            for i in range(6):
                nc.vector.memset(kT[i][:, M:M + 64], 0.0)
            aoT = [pp.tile([128, M], BF, name=f"aoT{i}", tag=f"aoT{i}") for i in range(6)]
            # v per frame/token-tile, natural token-major layout [tok, chan]
            vt = [[pp.tile([128, C], BF, name=f"v{f}_{j}", tag=f"v{f}_{j}")
                   for j in range(2)] for f in range(F)]
            # pair-selector blocks for the denominator matmuls: block p
            # (cols 6p:6p+6) is all-ones in column p, zero elsewhere, so a
            # single N=394 matmul with rhs = both heads of pair p writes
            # their key-sums into row p of a shared [6, 394] PSUM tile
            # (other rows accumulate +0).
            onesel = pp.tile([128, 6 * 6], BF, name="onesel", tag="onesel")
            nc.vector.memset(onesel[:, :], 0.0)
            for p in range(6):
                nc.vector.memset(onesel[:, p * 6 + p:p * 6 + p + 1], 1.0)

            # ---- phases 1-4, m-chunk pipelined: weight DMAs stream on the
            # otherwise-idle GpSimd queue (column-chunk-major so the qk
            # matmuls of chunk 0 can start after ~1/3 of qkv_w has landed)
            # while x loads/transposes and adapter/qk/v compute proceed per
            # 512-token chunk on sync/PE/ACT/DVE. This overlaps the whole
            # weight-load latency with compute instead of serializing.
            # chunk-0 x tiles are emitted before any weight loads so the
            # x casts/transposes head every engine queue (weight casts
            # otherwise block them in-order and delay the first matmul)
            pst_ctx = tc.tile_pool(name="pst", bufs=2, space="PSUM")
            pst = pst_ctx.__enter__()

            def emit_x_tile(mt):
                tb, tsz = MTILES[mt]
                xn = sp.tile([128, C], FP, bufs=4, name=f"xn{mt}", tag="wstg")
                nc.sync.dma_start(xn[0:tsz, :], xf[tb:tb + tsz, :])
                xb = sp.tile([128, C], BF, bufs=3, name=f"xb{mt}", tag="xb")
                nc.vector.tensor_copy(xb[0:tsz, :], xn[0:tsz, :])
                for ct in range(6):
                    pt = pst.tile([128, 128], BF, name=f"pt{mt}_{ct}", tag="pt")
                    nc.tensor.transpose(pt[:, 0:tsz],
                                        xb[0:tsz, ct * 128:(ct + 1) * 128],
                                        ident[0:tsz, 0:tsz])
                    nc.scalar.activation(xT[ct][:, tb:tb + tsz],
                                         pt[:, 0:tsz], AT.Copy)

            for mt in (0, 1, 2, 3):
                emit_x_tile(mt)

            _castn = [0]

            def stage_cast(dst, cb, csz, p, src_ap, name, q=None):
                stg = sp.tile([128, C], FP, bufs=4,
                              name=f"stg_{name}_{cb}", tag="wstg")
                # spread DMA issue between the sync and gpsimd queues
                # (issue costs ~0.7us each; one queue would serialize the
                # whole weight stream), and alternate the cast between DVE
                # and ACT
                if q is None:
                    q = nc.sync if _castn[0] % 2 == 0 else nc.gpsimd
                q.dma_start(stg[0:p, 0:csz], src_ap[:, cb:cb + csz])
                if _castn[0] % 2 == 0:
                    nc.vector.tensor_copy(dst[:, cb:cb + csz], stg[0:p, 0:csz])
                else:
                    nc.scalar.copy(dst[:, cb:cb + csz], stg[0:p, 0:csz])
                _castn[0] += 1

            aw1 = [pp.tile([128, HADP], BF, name=f"aw1_{k}", tag=f"aw1_{k}")
                   for k in range(6)]
            aw2 = [pp.tile([128, C], BF, name="aw2_0", tag="aw2_0"),
                   pp.tile([64, C], BF, name="aw2_1", tag="aw2_1")]
            qkvw = [pp.tile([128, 3 * C], BF, name=f"qkvw{k}", tag=f"qkvw{k}")
                    for k in range(6)]
            projw = [pp.tile([128, C], BF, name=f"projw{k}", tag=f"projw{k}")
                     for k in range(6)]

            # adapter biases first (needed ~5us in); packed column loads
            # (one DMA per bias tensor instead of one per 128-column)
            b1c_t = pp.tile([128, 2], FP, name="b1c", tag="b1c")
            nc.gpsimd.dma_start(b1c_t[:, 0:1], ab1_e[0:128][:, None])
            nc.gpsimd.dma_start(b1c_t[0:64, 1:2], ab1_e[128:HADP][:, None])
            b1c = [b1c_t[:, 0:1], b1c_t[0:64, 1:2]]
            b2c_t = pp.tile([128, 6], FP, name="b2c", tag="b2c")
            nc.gpsimd.dma_start(
                b2c_t[:, :], ab2_e[:].rearrange("(a p) -> p a", p=128))
            b2c = [b2c_t[:, i:i + 1] for i in range(6)]
            # qkv biases: first 12 columns are q/k (per-partition columns),
            # v bias needs the row-broadcast layout
            qkc_t = pp.tile([128, NH], FP, name="qkc", tag="qkc")
            nc.gpsimd.dma_start(
                qkc_t[:, :], qkvb_e[0:1536].rearrange("(a p) -> p a", p=128))
            qkbc = [qkc_t[:, i:i + 1] for i in range(12)]
            vbb = pp.tile([128, C], FP, name="vbb", tag="vbb")
            pbb = pp.tile([128, C], FP, name="pbb", tag="pbb")

            def load_qkvw_cc(cc):
                for k in range(6):
                    stage_cast(qkvw[k], cc * C, C,
                               128, qkvw_e[k * 128:(k + 1) * 128, :],
                               f"qkvw{k}")

            # adapter weights first (first compute consumer; small), then
            # the q/k/v weight column-chunks in first-use order
            for k in range(6):
                stage_cast(aw1[k], 0, HADP, 128, aw1_e[k * 128:(k + 1) * 128, :],
                           f"aw1_{k}", q=nc.gpsimd)
            stage_cast(aw2[0], 0, C, 128, aw2_e[0:128, :], "aw2_0", q=nc.gpsimd)
            stage_cast(aw2[1], 0, C, 64, aw2_e[128:HADP, :], "aw2_1",
                       q=nc.gpsimd)
            load_qkvw_cc(0)
            load_qkvw_cc(1)
            load_qkvw_cc(2)

            CHUNK_MTILES = {0: [0, 1, 2, 3], 1: [4, 5, 6, 7],
                            2: [8, 9, 10, 11], 3: [12]}
            CHUNK_FRAMES = {0: [0, 1], 1: [2, 3, 4], 2: [5, 6], 3: [7]}

            with pst_ctx_closer(pst_ctx), \
                 tc.tile_pool(name="psA", bufs=4, space="PSUM") as psA:
                hT = [sp.tile([128, M], BF, bufs=1, name="hT0", tag="hT0"),
                      sp.tile([64, M], BF, bufs=1, name="hT1", tag="hT1")]
                for c, (mb, csz) in enumerate(MCHUNKS):
                    # adapter for this chunk (m-local).  x tiles for chunk
                    # c+1 are emitted right after the adapter so their
                    # DMAs/casts/transposes prefetch during this chunk's
                    # qk/v compute instead of stalling the next chunk.
                    for ht, (hb, hsz) in enumerate([(0, 128), (128, 64)]):
                        ps = psA.tile([128, 512], FP, name=f"psh{ht}_{mb}",
                                      tag="psA")
                        for kt in range(6):
                            nc.tensor.matmul(ps[0:hsz, 0:csz],
                                             aw1[kt][:, hb:hb + hsz],
                                             xT[kt][:, mb:mb + csz],
                                             start=(kt == 0), stop=(kt == 5))
                        nc.scalar.activation(hT[ht][:, mb:mb + csz],
                                             ps[0:hsz, 0:csz],
                                             AT.Identity, bias=b1c[ht][:, :])
                    for ct in range(6):
                        ps = psA.tile([128, 512], FP, name=f"psx{ct}_{mb}",
                                      tag="psA")
                        for kt, ksz in enumerate([128, 64]):
                            nc.tensor.matmul(ps[:, 0:csz],
                                             aw2[kt][0:ksz, ct * 128:(ct + 1) * 128],
                                             hT[kt][0:ksz, mb:mb + csz],
                                             start=(kt == 0), stop=(kt == 1))
                        nc.vector.scalar_tensor_tensor(
                            out=xT[ct][:, mb:mb + csz], in0=ps[:, 0:csz],
                            scalar=b2c[ct][:, :], in1=xT[ct][:, mb:mb + csz],
                            op0=OP.add, op1=OP.add)
                    if c + 1 < len(MCHUNKS):
                        for mt in CHUNK_MTILES[c + 1]:
                            emit_x_tile(mt)
                    # qk for this chunk
                    for ot in range(12):
                        dst = qT[ot] if ot < 6 else kT[ot - 6]
                        ps = psA.tile([128, 512], FP, name=f"psqk{ot}_{mb}",
                                      tag="psA")
                        for kt in range(6):
                            nc.tensor.matmul(ps[:, 0:csz],
                                             qkvw[kt][:, ot * 128:(ot + 1) * 128],
                                             xT[kt][:, mb:mb + csz],
                                             start=(kt == 0), stop=(kt == 5))
                        nc.scalar.activation(dst[:, mb:mb + csz], ps[:, 0:csz],
                                             AT.Identity, bias=qkbc[ot][:, :])
                    if c == 0:
                        nc.gpsimd.dma_start(
                            vbb[:, :],
                            qkvb_e[2 * C:3 * C][None, :].broadcast_to((128, C)))
                    # v for the frames fully inside chunks <= c
                    for f in CHUNK_FRAMES[c]:
                        for jt, (jb, jsz) in enumerate(JTILES):
                            for half in range(2):
                                ps = psA.tile([128, 512], FP,
                                              name=f"psv{f}_{jt}_{half}",
                                              tag="psA")
                                for kt in range(6):
                                    nc.tensor.matmul(
                                        ps[0:jsz, 0:384],
                                        xT[kt][:, f * N + jb: f * N + jb + jsz],
                                        qkvw[kt][:, 1536 + half * 384:
                                                 1536 + (half + 1) * 384],
                                        start=(kt == 0), stop=(kt == 5))
                                nc.vector.tensor_tensor(
                                    out=vt[f][jt][0:jsz,
                                                  half * 384:(half + 1) * 384],
                                    in0=ps[0:jsz, 0:384],
                                    in1=vbb[0:jsz, half * 384:(half + 1) * 384],
                                    op=OP.add)
                    if c == 1:
                        nc.gpsimd.dma_start(
                            pbb[:, :],
                            projb_e[:][None, :].broadcast_to((128, C)))
                        for k in range(6):
                            stage_cast(projw[k], 0, C,
                                       128, projw_e[k * 128:(k + 1) * 128, :],
                                       f"projw{k}")

            # ---- phase 5: attention frame loop, 9 score units per
            # frame (3 'E' tiles: both key chunks of even heads (2a, 2a+6)
            # in one 2-bank tile at in-bank offsets — legal for base-0
            # operands; 6 'O' tiles: one key chunk of odd heads
            # (2c+1, 2c+7) at bank-aligned offsets — base-64 matmuls must
            # write bank offset 0 and banks must not mix base-64/base-0
            # writers), one batched exp per unit.
            # av matmuls for pair p run IN-frame as soon as its three es
            # units exist (unnormalized, evicted to SBUF bf16 on DVE); the
            # normalization multiply trails into frame f+1 after the
            # reciprocal DRAM bounce, so neither the PE nor the frame
            # cadence ever waits on the bounce.
            # den: one N=394 matmul per (unit, key chunk) — lhsT is an
            # all-ones column selecting one row of a [6, 394] PSUM tile
            # (row a = even pair (2a, 2a+6), row 3+c = odd pair
            # (2c+1, 2c+7); cols 0:197 first head, 197:394 second).
            def fk_of(f, h):
                if h < 2:
                    return max(f - 1, 0)
                if h < 4:
                    return min(f + 1, F - 1)
                return f

            # proj tile m is emitted during the frame after the last frame
            # its token rows touch; PROJ_AT[f] = tiles ready in frame f
            PROJ_AT = {2: [0], 3: [1, 2], 4: [3], 5: [4, 5], 6: [6],
                       7: [7, 8]}
            UNITS = [("E", 0), ("O", 0, 0), ("O", 0, 1),
                     ("E", 1), ("O", 1, 0), ("O", 1, 1),
                     ("E", 2), ("O", 2, 0), ("O", 2, 1)]
            # unit index -> av pairs whose es units are all ready
            AV_AT = {3: [0], 4: [3], 6: [1], 7: [4]}

            def unit_heads(u):
                # [(head, jt, bank_col)] covered by unit u
                if u[0] == "E":
                    a = u[1]
                    return [(2 * a + 6 * s, jt, s * 512 + jt * N)
                            for s in range(2) for jt in range(2)]
                _, c, jt = u
                return [(2 * c + 1 + 6 * s, jt, s * 512) for s in range(2)]

            with tc.tile_pool(name="psT", bufs=1, space="PSUM") as psT, \
                 tc.tile_pool(name="drp", bufs=2, space="DRAM") as drp:

                es_all = {}       # es_all[(f, u)] -> es tile
                aou_all = {}      # aou_all[(f, p)] -> [128, 197] bf16 unnorm
                rec128_all = {}   # rec128_all[f] -> [128, 6*197] fp32
                den_all = {}      # den_all[f] -> [6, 512] PSUM
                nden = {}

                def es_get(gf, h, jt):
                    if h % 2 == 0:
                        a = (h // 2) % 3
                        return (es_all[(gf, ("E", a))],
                                (0 if h < 6 else 2 * N) + jt * N)
                    c = ((h - 1) // 2) % 3
                    return es_all[(gf, ("O", c, jt))], (0 if h < 7 else N)

                def emit_scores(f, i, u):
                    esz = 4 * N if u[0] == "E" else 2 * N
                    stt = psT.tile([128, 1024], FP, bufs=2,
                                   name=f"st{f}_{i}", tag="st")
                    for h, jt, bcol in unit_heads(u):
                        jb = JTILES[jt][0]
                        pb = 64 * (h % 2)
                        g = h // 2
                        fk = fk_of(f, h)
                        # always a full 128-wide kT window so all PSUM
                        # partitions the batched exp reads are written
                        # (the jt=1 tail rows hit the next frame's keys /
                        # the zero pad; never read downstream)
                        nc.tensor.matmul(
                            stt[0:128, bcol:bcol + N],
                            kT[g][pb:pb + 64,
                                  fk * N + jb: fk * N + jb + 128],
                            qT[g][pb:pb + 64, f * N:(f + 1) * N],
                            start=True, stop=True)
                    e = sp.tile([128, esz], BF,
                                bufs=(4 if u[0] == "E" else 7),
                                name=f"e{f}_{i}",
                                tag=("eE" if u[0] == "E" else "eO"))
                    nc.scalar.activation(
                        e[0:128, :].rearrange("p (b q) -> p b q", b=2),
                        stt[0:128, 0:1024].rearrange(
                            "p (b q) -> p b q", b=2)[:, :, 0:esz // 2],
                        AT.Exp, scale=SCALE)
                    es_all[(f, u)] = e

                def emit_den(f, u):
                    if f not in den_all:
                        den_all[f] = psT.tile([6, 512], FP, bufs=1,
                                              name=f"den{f}", tag="den")
                        nden[f] = 0
                    dt = den_all[f]
                    e = es_all[(f, u)]
                    if u[0] == "E":
                        a = u[1]
                        for jt, (jb, jsz) in enumerate(JTILES):
                            nc.tensor.matmul(
                                dt[0:6, 0:2 * N],
                                onesel[0:jsz, a * 6:(a + 1) * 6],
                                e[0:jsz, :].rearrange(
                                    "p (hh q) -> p hh q",
                                    hh=2)[:, :, jt * N:(jt + 1) * N],
                                start=(nden[f] == 0), stop=(nden[f] == 11))
                            nden[f] += 1
                    else:
                        c, jt = u[1], u[2]
                        jsz = JTILES[jt][1]
                        nc.tensor.matmul(
                            dt[0:6, 0:2 * N],
                            onesel[0:jsz, (3 + c) * 6:(4 + c) * 6],
                            e[0:jsz, 0:2 * N],
                            start=(nden[f] == 0), stop=(nden[f] == 11))
                        nden[f] += 1

                def emit_av(f, p):
                    av_t = psT.tile([128, 512], FP, bufs=2,
                                    name=f"av{f}_{p}", tag="av")
                    for hi in range(2):
                        h = 2 * p + hi
                        fk = fk_of(f, h)
                        for jt, (jb, jsz) in enumerate(JTILES):
                            e, cb = es_get(f, h, jt)
                            nc.tensor.matmul(
                                av_t[hi * 64:(hi + 1) * 64, 0:N],
                                vt[fk][jt][0:jsz, h * HD:(h + 1) * HD],
                                e[0:jsz, cb:cb + N],
                                start=(jt == 0), stop=(jt == 1))
                    ao = sp.tile([128, N], BF, bufs=8, name=f"aou{f}_{p}",
                                 tag="aou")
                    nc.vector.tensor_copy(ao[:, :], av_t[:, 0:N])
                    aou_all[(f, p)] = ao

                def emit_norm(f, p):
                    nc.vector.tensor_tensor(
                        out=aoT[p][:, f * N:(f + 1) * N],
                        in0=aou_all.pop((f, p))[:, :],
                        in1=rec128_all[f][:, p * N:(p + 1) * N],
                        op=OP.mult)

                def emit_bounce(f):
                    rec6 = sp.tile([6, 2 * N], FP, bufs=2, name=f"rcp{f}",
                                   tag="rec6")
                    nc.vector.reciprocal(rec6[:, :], den_all[f][:, 0:2 * N])
                    dr6 = drp.tile([6, 2 * N], FP, name=f"dr6_{f}", tag="dr6")
                    nc.gpsimd.dma_start(dr6[:, :], rec6[:, :])
                    rec128 = sp.tile([128, 6 * N], FP, bufs=2,
                                     name=f"rec128_{f}", tag="rec128")
                    for b in range(2):
                        # b=0: rows 0:64 <- even heads = dr6 rows 0:3;
                        # b=1: rows 64:128 <- odd heads = dr6 rows 3:6.
                        # rec128 col-block p maps to (half a, den row r)
                        # with p = a*3 + r
                        for a in range(2):
                            src = dr6[3 * b:3 * b + 3,
                                      a * N:(a + 1) * N][None]
                            nc.gpsimd.dma_start(
                                rec128[b * 64:(b + 1) * 64,
                                       a * 3 * N:(a + 1) * 3 * N].rearrange(
                                    "p (r q) -> p r q", r=3),
                                src.broadcast_to((64, 3, N)))
                    rec128_all[f] = rec128

                def emit_proj(mt):
                    mb, msz = MTILES[mt]
                    pt = psT.tile([128, 1024], FP, bufs=2,
                                  name=f"psp{mt}", tag="st")
                    for half in range(2):
                        for kt in range(6):
                            nc.tensor.matmul(
                                pt[0:msz, half * 512:half * 512 + 384],
                                aoT[kt][:, mb:mb + msz],
                                projw[kt][:, half * 384:(half + 1) * 384],
                                start=(kt == 0), stop=(kt == 5))
                    osb = sp.tile([128, C], FP, bufs=2,
                                  name=f"osb{mt}", tag="osb")
                    nc.vector.tensor_tensor(
                        out=osb[0:msz, :].rearrange("p (b c) -> p b c", b=2),
                        in0=pt[0:msz, 0:1024].rearrange(
                            "p (b c) -> p b c", b=2)[:, :, 0:384],
                        in1=pbb[0:msz, :].rearrange("p (b c) -> p b c", b=2),
                        op=OP.add)
                    nc.sync.dma_start(outf[mb:mb + msz, :], osb[0:msz, :])

                for f in range(F):
                    for i, u in enumerate(UNITS):
                        emit_scores(f, i, u)
                        if i >= 1:
                            emit_den(f, UNITS[i - 1])
                        for pq in AV_AT.get(i, ()):
                            emit_av(f, pq)
                        if f >= 1:
                            if i == 2:
                                for q in range(3):
                                    emit_norm(f - 1, q)
                            if i == 4:
                                for q in range(3, 6):
                                    emit_norm(f - 1, q)
                            if i == 5 and f in PROJ_AT:
                                emit_proj(PROJ_AT[f][0])
                            if i == 7 and len(PROJ_AT.get(f, ())) > 1:
                                emit_proj(PROJ_AT[f][1])
                    emit_den(f, UNITS[8])
                    emit_av(f, 2)
                    emit_av(f, 5)
                    emit_bounce(f)

                # epilogue: frame 7 normalization trails the bounce; proj
                # tile 10's rows 1280:1344 only need frames <= 6, so those
                # matmuls bridge the frame-7 reciprocal-bounce latency
                mb10 = MTILES[10][0]
                pt10 = psT.tile([128, 1024], FP, bufs=2,
                                name="psp10", tag="st")
                for half in range(2):
                    for kt in range(6):
                        nc.tensor.matmul(
                            pt10[0:64, half * 512:half * 512 + 384],
                            aoT[kt][:, mb10:mb10 + 64],
                            projw[kt][:, half * 384:(half + 1) * 384],
                            start=(kt == 0), stop=(kt == 5))
                emit_proj(9)
                for q in range(6):
                    emit_norm(7, q)
                for half in range(2):
                    for kt in range(6):
                        nc.tensor.matmul(
                            pt10[64:128, half * 512:half * 512 + 384],
                            aoT[kt][:, mb10 + 64:mb10 + 128],
                            projw[kt][:, half * 384:(half + 1) * 384],
                            start=(kt == 0), stop=(kt == 5))
                osb10 = sp.tile([128, C], FP, bufs=2, name="osb10", tag="osb")
                nc.vector.tensor_tensor(
                    out=osb10[0:128, :].rearrange("p (b c) -> p b c", b=2),
                    in0=pt10[0:128, 0:1024].rearrange(
                        "p (b c) -> p b c", b=2)[:, :, 0:384],
                    in1=pbb[0:128, :].rearrange("p (b c) -> p b c", b=2),
                    op=OP.add)
                nc.sync.dma_start(outf[mb10:mb10 + 128, :], osb10[0:128, :])
                emit_proj(11)
                emit_proj(12)

    nc.compile()
    return nc


def _get_nc():
    if "nc" not in _CACHE:
        _CACHE["nc"] = _build()
    return _CACHE["nc"]


def _in_maps(inputs):
    x = np.ascontiguousarray(np.asarray(inputs["x"], np.float32))
    w = {k: np.ascontiguousarray(np.asarray(inputs[k], np.float32))
         for k in ("a_w1", "a_b1", "a_w2", "a_b2", "qkv_w", "qkv_b",
                   "proj_w", "proj_b")}
    maps = []
    for i in range(NCORES):
        m = {"x": x[i * F:(i + 1) * F]}
        m.update(w)
        maps.append(m)
    return maps


def kernel(**inputs):
    from concourse.bass_utils import run_bass_kernel_spmd
    nc = _get_nc()
    res = run_bass_kernel_spmd(nc, _in_maps(inputs), core_ids=list(range(NCORES)))
    return np.concatenate([res.results[i]["out"] for i in range(NCORES)], axis=0)


def run_traced(inputs, **kwargs):
    """Test harness helper: run with NTFF profiling, return (output, results)."""
    from concourse.bass_utils import run_bass_kernel_spmd
    nc = _get_nc()
    res = run_bass_kernel_spmd(nc, _in_maps(inputs),
                               core_ids=list(range(NCORES)), trace=True, **kwargs)
    out = np.concatenate([res.results[i]["out"] for i in range(NCORES)], axis=0)
    return out, res


if __name__ == "__main__":
    # quick compile check
    _build()
    print("compile OK")

